# revision 3
# baseline (speedup 1.0000x reference)
"""GCN layer (nn_GCNLayer_72224170050097) as a Bass/Tile kernel on 8 TRN2 NeuronCores.

Math (reference):
    a_hat = adj + I
    d = rowsum(a_hat) ** -0.5
    out = (a_hat * d[:, None] * d[None, :]) @ x @ W.T + b

Sharding: 1D row-parallel over N=8192 (1024 rows per core).

Per-core device program (SPMD, one NEFF, collective for the degree vector):
  - The core's row-block of a_hat is staged host-side as its TRANSPOSE in bf16
    ("adjT", [8192, 1024], j-major) so the 8192-long contraction dim lands on
    SBUF partitions with contiguous DMA.  The j index is consumed in the
    permutation j = p*64 + c (p = partition, c = chunk) which is baked into
    both adjT and x host-side - the contraction is permutation invariant.
  - adjT is DMA'd once and cached entirely in SBUF (16 MB bf16 < 24 MB SBUF).
  - Degree pass: ones^T @ adjT chunk-by-chunk on the TensorEngine, pipelined
    under the DMA.  d_local = 1/sqrt(deg) -> 4 KB AllGather -> full d.
  - Main pass: y^T = (d*x)^T @ adjT accumulated in PSUM over the 64 chunks.
  - Epilogue: y^T * d_row (this core's d), W matmul, + bias, all on-chip.
  - Output is out^T [128, 1024] fp32 per core; the host transposes/concats.
"""

import sys

if "/opt/trn_rl_repo" not in sys.path:
    sys.path.insert(0, "/opt/trn_rl_repo")

import numpy as np
import ml_dtypes

import concourse.bass as bass  # noqa: F401  (bass must be imported before tile)
import concourse.mybir as mybir
import concourse.tile as tile
from concourse import bacc
from concourse.bass_utils import run_bass_kernel_spmd

N = 8192
D = 128
NCORES = 8
NB = N // NCORES  # 1024 rows per core
P = 128
C = N // P  # 64 chunks of the contraction dim
H = NB // 512  # 2 free-dim halves of 512

dt = mybir.dt
BF16 = ml_dtypes.bfloat16

_CACHE = {}


def build_nc():
    nc = bacc.Bacc(
        "TRN2",
        target_bir_lowering=False,
        debug=False,
        num_devices=NCORES,
    )
    adjT = nc.dram_tensor("adjT", [N, NB], dt.bfloat16, kind="ExternalInput").ap()
    xin = nc.dram_tensor("xin", [N, D], dt.bfloat16, kind="ExternalInput").ap()
    wt = nc.dram_tensor("wt", [D, D], dt.bfloat16, kind="ExternalInput").ap()
    bias = nc.dram_tensor("bias", [D, 1], dt.float32, kind="ExternalInput").ap()
    outT = nc.dram_tensor("outT", [D, NB], dt.float32, kind="ExternalOutput").ap()

    with tile.TileContext(nc) as tc:
        with (
            tc.tile_pool(name="at", bufs=C) as atpool,
            tc.tile_pool(name="sb", bufs=1) as sb,
            tc.tile_pool(name="ps", bufs=1, space="PSUM") as ps,
            tc.tile_pool(name="dram", bufs=1, space="DRAM") as dram,
        ):
            # j = p*64 + c  (partition-major permutation of the contraction dim)
            adjT3 = adjT.rearrange("(p c) i -> p c i", c=C)
            xin3 = xin.rearrange("(p c) f -> p c f", c=C)

            ones = sb.tile([P, 1], dt.bfloat16, tag="ones")
            nc.vector.memset(ones[:], 1.0)
            wts = sb.tile([D, D], dt.bfloat16, tag="wts")
            nc.sync.dma_start(wts[:], wt)
            bs = sb.tile([D, 1], dt.float32, tag="bs")
            nc.sync.dma_start(bs[:], bias)
            X = sb.tile([P, C, D], dt.bfloat16, tag="X")
            nc.sync.dma_start(X[:], xin3)

            # ---- degree pass, pipelined with the adjT DMA ----
            pdeg = [ps.tile([1, 512], dt.float32, tag=f"pdeg{h}", name=f"pdeg{h}") for h in range(H)]
            at_tiles = []
            for c in range(C):
                at = atpool.tile([P, NB], dt.bfloat16, tag="at")
                nc.sync.dma_start(at[:], adjT3[:, c, :])
                at_tiles.append(at)
                for h in range(H):
                    nc.tensor.matmul(
                        pdeg[h][:],
                        lhsT=ones[:],
                        rhs=at[:, h * 512 : (h + 1) * 512],
                        start=(c == 0),
                        stop=(c == C - 1),
                    )

            # d_local = 1/sqrt(deg)   (Rsqrt on ACT is banned for accuracy)
            sq = sb.tile([1, NB], dt.float32, tag="sq")
            dloc = sb.tile([1, NB], dt.float32, tag="dloc")
            for h in range(H):
                nc.scalar.activation(
                    sq[:, h * 512 : (h + 1) * 512],
                    pdeg[h][:],
                    mybir.ActivationFunctionType.Sqrt,
                )
            nc.vector.reciprocal(dloc[:], sq[:])

            dloc_d = dram.tile([1, NB], dt.float32)
            nc.sync.dma_start(dloc_d[:], dloc[:])
            dfull_d = dram.tile([NCORES, NB], dt.float32)
            nc.gpsimd.collective_compute(
                "AllGather",
                mybir.AluOpType.bypass,
                replica_groups=[list(range(NCORES))],
                ins=[dloc_d[:].opt()],
                outs=[dfull_d[:].opt()],
            )

            # full d in [p, c] layout (j = p*64 + c; dfull[k, i] is j = k*1024 + i)
            Dsb = sb.tile([P, C], dt.float32, tag="Dsb")
            nc.sync.dma_start(
                Dsb[:], dfull_d[:].rearrange("k (pp c) -> (k pp) c", c=C)
            )
            # this core's d broadcast over all partitions (for the y^T row scale)
            drep = sb.tile([P, NB], dt.float32, tag="drep")
            nc.gpsimd.dma_start(drep[:], dloc_d[:].to_broadcast([P, NB]))

            # XS = X * d_j  (bf16), in slabs so the main pass can start early
            XS = sb.tile([P, C, D], dt.bfloat16, tag="XS")
            SL = 8
            for s in range(C // SL):
                sl = slice(s * SL, (s + 1) * SL)
                nc.vector.tensor_tensor(
                    XS[:, sl, :],
                    X[:, sl, :],
                    Dsb[:, sl, None].to_broadcast([P, SL, D]),
                    mybir.AluOpType.mult,
                )

            # ---- main pass: y^T[f, i] accumulated over the 64 chunks ----
            py = [ps.tile([P, 512], dt.float32, tag=f"py{h}", name=f"py{h}") for h in range(H)]
            for c in range(C):
                for h in range(H):
                    nc.tensor.matmul(
                        py[h][:],
                        lhsT=XS[:, c, :],
                        rhs=at_tiles[c][:, h * 512 : (h + 1) * 512],
                        start=(c == 0),
                        stop=(c == C - 1),
                    )

            # y^T * d_row -> bf16 for the W matmul
            yt = sb.tile([P, NB], dt.bfloat16, tag="yt")
            for h in range(H):
                nc.vector.tensor_tensor(
                    yt[:, h * 512 : (h + 1) * 512],
                    py[h][:],
                    drep[:, h * 512 : (h + 1) * 512],
                    mybir.AluOpType.mult,
                )

            # z^T = W @ y^T  (lhsT = W.T staged host-side)
            pz = [ps.tile([P, 512], dt.float32, tag=f"pz{h}", name=f"pz{h}") for h in range(H)]
            for h in range(H):
                nc.tensor.matmul(
                    pz[h][:],
                    lhsT=wts[:],
                    rhs=yt[:, h * 512 : (h + 1) * 512],
                    start=True,
                    stop=True,
                )

            # + bias, copy to SBUF fp32, DMA out
            osb = sb.tile([D, NB], dt.float32, tag="osb")
            for h in range(H):
                nc.scalar.activation(
                    osb[:, h * 512 : (h + 1) * 512],
                    pz[h][:],
                    mybir.ActivationFunctionType.Identity,
                    bias=bs[:],
                    scale=1.0,
                )
            nc.sync.dma_start(outT, osb[:])

    nc.compile()
    return nc


def get_nc():
    if "nc" not in _CACHE:
        _CACHE["nc"] = build_nc()
    return _CACHE["nc"]


def make_in_maps(x, adj, W, b):
    x = np.asarray(x, dtype=np.float32)
    adj = np.asarray(adj, dtype=np.float32)
    W = np.asarray(W, dtype=np.float32)
    b = np.asarray(b, dtype=np.float32)

    xin16 = np.ascontiguousarray(x).astype(BF16)
    wt16 = np.ascontiguousarray(W.T).astype(BF16)
    bias32 = np.ascontiguousarray(b.reshape(D, 1))

    in_maps = []
    idx = np.arange(NB)
    for k in range(NCORES):
        blk = adj[k * NB : (k + 1) * NB, :]  # [NB, N]
        at16 = blk.T.astype(BF16)  # [N, NB], contiguous
        # bake the +I diagonal for this core's rows
        at16[k * NB + idx, idx] = (blk[idx, k * NB + idx] + 1.0).astype(BF16)
        in_maps.append({"adjT": at16, "xin": xin16, "wt": wt16, "bias": bias32})
    return in_maps


def kernel(**inputs) -> np.ndarray:
    nc = get_nc()
    in_maps = make_in_maps(
        inputs["x"], inputs["adj"], inputs["W"], inputs["b"]
    )
    res = run_bass_kernel_spmd(nc, in_maps, list(range(NCORES)))
    out = np.empty((N, D), dtype=np.float32)
    for k in range(NCORES):
        out[k * NB : (k + 1) * NB, :] = res.results[k]["outT"].T
    return out


# revision 9
# speedup vs baseline: 4225.6509x; 4225.6509x over previous
"""GCN layer (nn_GCNLayer_72224170050097) as a Bass/Tile kernel on 8 TRN2 NeuronCores.

Math (reference):
    a_hat = adj + I
    d = rowsum(a_hat) ** -0.5
    out = (a_hat * d[:, None] * d[None, :]) @ x @ W.T + b

Sharding: 1D row-parallel over N=8192 (1024 rows per core).

Per-core device program (SPMD, one NEFF, collective for the degree vector):
  - The core's row-block of a_hat is staged host-side as its TRANSPOSE in bf16
    ("adjT", [8192, 1024], j-major) so the 8192-long contraction dim lands on
    SBUF partitions with contiguous DMA.  The j index is consumed in the
    permutation j = p*64 + c (p = partition, c = chunk) which is baked into
    both adjT and x host-side - the contraction is permutation invariant.
  - adjT is DMA'd once and cached entirely in SBUF (16 MB bf16 < 24 MB SBUF).
  - Degree pass: ones^T @ adjT chunk-by-chunk on the TensorEngine, pipelined
    under the DMA.  d_local = 1/sqrt(deg) -> 4 KB AllGather -> full d.
  - Main pass: y^T = (d*x)^T @ adjT accumulated in PSUM over the 64 chunks.
  - Epilogue: y^T * d_row (this core's d), W matmul, + bias, all on-chip.
  - Output is out^T [128, 1024] fp32 per core; the host transposes/concats.
"""

import sys

if "/opt/trn_rl_repo" not in sys.path:
    sys.path.insert(0, "/opt/trn_rl_repo")

import numpy as np
import ml_dtypes

import concourse.bass as bass  # noqa: F401  (bass must be imported before tile)
import concourse.mybir as mybir
import concourse.tile as tile
from concourse import bacc
from concourse.bass_utils import run_bass_kernel_spmd

N = 8192
D = 128
NCORES = 8
NB = N // NCORES  # 1024 rows per core
P = 128
C = N // P  # 64 chunks of the contraction dim
H = NB // 512  # 2 free-dim halves of 512

dt = mybir.dt
BF16 = ml_dtypes.bfloat16

_CACHE = {}


def _emit_body(nc, pools, aps, rep):
    """One full kernel iteration.  rep only suffixes debug names; pools/tags
    are shared so repeated bodies (timing builds) recycle the same SBUF."""
    atpool, sb, ps, dram = pools
    adjT3, xin3, wt, bias, outT = aps
    r = f"_{rep}"
    G = 4  # chunks per DMA (1 MiB transfers)

    ones = sb.tile([P, 1], dt.bfloat16, tag="ones", name="ones" + r)
    nc.vector.memset(ones[:], 1.0)

    # ---- adjT DMA (16 x 1MiB) + degree pass pipelined under it ----
    pdeg = [
        ps.tile([1, 512], dt.float32, tag=f"pdeg{h}", name=f"pdeg{h}{r}")
        for h in range(H)
    ]
    at_tiles = []
    first_at_inst = None
    for g in range(C // G):
        at = atpool.tile([P, G, NB], dt.bfloat16, tag="at", name=f"at{g}{r}")
        dma_inst = nc.sync.dma_start(at[:], adjT3[:, g * G : (g + 1) * G, :])
        if first_at_inst is None:
            first_at_inst = dma_inst
        at_tiles.append(at)
        for q in range(G):
            c = g * G + q
            for h in range(H):
                nc.tensor.matmul(
                    pdeg[h][:],
                    lhsT=ones[:],
                    rhs=at[:, q, h * 512 : (h + 1) * 512],
                    start=(c == 0),
                    stop=(c == C - 1),
                )

    # raw degrees -> DRAM -> AllGather (rsqrt happens after, on 128 lanes)
    degloc = sb.tile([1, NB], dt.float32, tag="degloc", name="degloc" + r)
    for h in range(H):
        nc.scalar.copy(degloc[:, h * 512 : (h + 1) * 512], pdeg[h][:])
    degloc_d = dram.tile([1, NB], dt.float32, tag="degloc_d", name="degloc_d" + r)
    nc.sync.dma_start(degloc_d[:], degloc[:])
    degfull_d = dram.tile(
        [NCORES, NB], dt.float32, tag="degfull_d", name="degfull_d" + r
    )
    nc.gpsimd.collective_compute(
        "AllGather",
        mybir.AluOpType.bypass,
        replica_groups=[list(range(NCORES))],
        ins=[degloc_d[:].opt()],
        outs=[degfull_d[:].opt()],
    )

    # X / W / bias DMAs land during the degree tail + collective window
    X = sb.tile([P, C, D], dt.bfloat16, tag="X", name="X" + r)
    nc.sync.dma_start(X[:], xin3)
    wts = sb.tile([D, D], dt.bfloat16, tag="wts", name="wts" + r)
    nc.sync.dma_start(wts[:], wt)
    bs = sb.tile([D, 1], dt.float32, tag="bs", name="bs" + r)
    nc.sync.dma_start(bs[:], bias)

    # this core's d = 1/sqrt(deg) for the output row scale; off the critical
    # path (only the epilogue needs it).  Rsqrt on ACT is banned for accuracy.
    sq = sb.tile([1, NB], dt.float32, tag="sq", name="sq" + r)
    dloc = sb.tile([1, NB], dt.float32, tag="dloc", name="dloc" + r)
    for h in range(H):
        nc.scalar.activation(
            sq[:, h * 512 : (h + 1) * 512],
            pdeg[h][:],
            mybir.ActivationFunctionType.Sqrt,
        )
    nc.vector.reciprocal(dloc[:], sq[:])
    dloc_d = dram.tile([1, NB], dt.float32, tag="dloc_d", name="dloc_d" + r)
    nc.sync.dma_start(dloc_d[:], dloc[:])
    drep = sb.tile([P, NB], dt.float32, tag="drep", name="drep" + r)
    nc.gpsimd.dma_start(drep[:], dloc_d[:].to_broadcast([P, NB]))

    # full d in [p, c] layout (j = p*64 + c; degfull[k, i] is j = k*1024 + i):
    # wide 128-lane rsqrt right after the collective
    Dg = sb.tile([P, C], dt.float32, tag="Dg", name="Dg" + r)
    nc.sync.dma_start(Dg[:], degfull_d[:].rearrange("k (pp c) -> (k pp) c", c=C))
    Dsq = sb.tile([P, C], dt.float32, tag="Dsq", name="Dsq" + r)
    nc.scalar.activation(Dsq[:], Dg[:], mybir.ActivationFunctionType.Sqrt)
    Dsb = sb.tile([P, C], dt.float32, tag="Dsb", name="Dsb" + r)
    nc.vector.reciprocal(Dsb[:], Dsq[:])

    # XS = X * d_j  (bf16), in slabs so the main pass can start early
    XS = sb.tile([P, C, D], dt.bfloat16, tag="XS", name="XS" + r)
    SL = 8
    for s in range(C // SL):
        sl = slice(s * SL, (s + 1) * SL)
        nc.vector.tensor_tensor(
            XS[:, sl, :],
            X[:, sl, :],
            Dsb[:, sl, None].to_broadcast([P, SL, D]),
            mybir.AluOpType.mult,
        )

    # ---- main pass: y^T[f, i] accumulated over the 64 chunks ----
    py = [
        ps.tile([P, 512], dt.float32, tag=f"py{h}", name=f"py{h}{r}")
        for h in range(H)
    ]
    for c in range(C):
        for h in range(H):
            nc.tensor.matmul(
                py[h][:],
                lhsT=XS[:, c, :],
                rhs=at_tiles[c // G][:, c % G, h * 512 : (h + 1) * 512],
                start=(c == 0),
                stop=(c == C - 1),
            )

    # y^T * d_row -> bf16 for the W matmul
    yt = sb.tile([P, NB], dt.bfloat16, tag="yt", name="yt" + r)
    for h in range(H):
        nc.vector.tensor_tensor(
            yt[:, h * 512 : (h + 1) * 512],
            py[h][:],
            drep[:, h * 512 : (h + 1) * 512],
            mybir.AluOpType.mult,
        )

    # z^T = W @ y^T  (lhsT = W.T staged host-side)
    pz = [
        ps.tile([P, 512], dt.float32, tag=f"pz{h}", name=f"pz{h}{r}")
        for h in range(H)
    ]
    for h in range(H):
        nc.tensor.matmul(
            pz[h][:],
            lhsT=wts[:],
            rhs=yt[:, h * 512 : (h + 1) * 512],
            start=True,
            stop=True,
        )

    # + bias, copy to SBUF fp32, DMA out
    osb = sb.tile([D, NB], dt.float32, tag="osb", name="osb" + r)
    for h in range(H):
        nc.scalar.activation(
            osb[:, h * 512 : (h + 1) * 512],
            pz[h][:],
            mybir.ActivationFunctionType.Identity,
            bias=bs[:],
            scale=1.0,
        )
    out_inst = nc.sync.dma_start(outT, osb[:])
    return first_at_inst, out_inst


def build_nc(reps=None):
    """reps=None -> single body (production).  reps=R -> body statically
    unrolled R times in straight line (timing builds; collectives stay in
    straight-line order, which NRT requires)."""
    nc = bacc.Bacc(
        "TRN2",
        target_bir_lowering=False,
        debug=False,
        num_devices=NCORES,
    )
    adjT = nc.dram_tensor("adjT", [N, NB], dt.bfloat16, kind="ExternalInput").ap()
    xin = nc.dram_tensor("xin", [N, D], dt.bfloat16, kind="ExternalInput").ap()
    wt = nc.dram_tensor("wt", [D, D], dt.bfloat16, kind="ExternalInput").ap()
    bias = nc.dram_tensor("bias", [D, 1], dt.float32, kind="ExternalInput").ap()
    outT = nc.dram_tensor("outT", [D, NB], dt.float32, kind="ExternalOutput").ap()

    with tile.TileContext(nc) as tc:
        with (
            tc.tile_pool(name="at", bufs=C // 4) as atpool,
            tc.tile_pool(name="sb", bufs=1) as sb,
            tc.tile_pool(name="ps", bufs=1, space="PSUM") as ps,
            tc.tile_pool(name="dram", bufs=1, space="DRAM") as dram,
        ):
            adjT3 = adjT.rearrange("(p c) i -> p c i", c=C)
            xin3 = xin.rearrange("(p c) f -> p c f", c=C)
            pools = (atpool, sb, ps, dram)
            aps = (adjT3, xin3, wt, bias, outT)
            prev_out = None
            for rep in range(reps or 1):
                first, out = _emit_body(nc, pools, aps, rep)
                if prev_out is not None:
                    # serialize reps so a timing slope measures single-shot
                    # latency instead of pipelined throughput
                    bass._add_dep_helper(
                        first.ins, prev_out.ins, sync=True,
                        reason="timing: serialize reps",
                    )
                prev_out = out

    nc.compile()
    return nc


def get_nc():
    if "nc" not in _CACHE:
        _CACHE["nc"] = build_nc()
    return _CACHE["nc"]


def make_in_maps(x, adj, W, b):
    x = np.asarray(x, dtype=np.float32)
    adj = np.asarray(adj, dtype=np.float32)
    W = np.asarray(W, dtype=np.float32)
    b = np.asarray(b, dtype=np.float32)

    xin16 = np.ascontiguousarray(x).astype(BF16)
    wt16 = np.ascontiguousarray(W.T).astype(BF16)
    bias32 = np.ascontiguousarray(b.reshape(D, 1))

    in_maps = []
    idx = np.arange(NB)
    for k in range(NCORES):
        blk = adj[k * NB : (k + 1) * NB, :]  # [NB, N]
        at16 = blk.T.astype(BF16)  # [N, NB], contiguous
        # bake the +I diagonal for this core's rows
        at16[k * NB + idx, idx] = (blk[idx, k * NB + idx] + 1.0).astype(BF16)
        in_maps.append({"adjT": at16, "xin": xin16, "wt": wt16, "bias": bias32})
    return in_maps


def kernel(**inputs) -> np.ndarray:
    nc = get_nc()
    in_maps = make_in_maps(inputs["x"], inputs["adj"], inputs["W"], inputs["b"])
    res = run_bass_kernel_spmd(nc, in_maps, list(range(NCORES)))
    out = np.empty((N, D), dtype=np.float32)
    for k in range(NCORES):
        out[k * NB : (k + 1) * NB, :] = res.results[k]["outT"].T
    return out
